# revision 5
# baseline (speedup 1.0000x reference)
"""Trainium2 Bass kernel for the pairwise concordance-index loss.

reference:
    loss = sum_{i<j, f_i=f_j=1} relu((p_i-p_j)(t_i-t_j)) / 100 / n_pairs

Math:
  M[i,j] = f_i f_j (p_i-p_j)(t_i-t_j) = A^T B, rank 4:
      A = [f*u, f, f*p, f*t],  B = [f, f*u, -f*t, -f*p],  u = p*t
  (flags fold in because relu(f_i f_j x) = f_i f_j relu(x) for 0/1 flags)
  sum relu(M) = 0.5*(sum M + sum |M|); sum M has an O(B) closed form done
  on the host in fp64; sum |M| is the O(B^2) part done on device.

Device decomposition (8 cores, identical program, data-sharded):
  64 row-blocks of 128 rows. Core k owns row-blocks 8k..8k+7. For each
  owned block a it processes the cyclic column-offsets e=0..32 (columns
  128a+128e mod 8192). e=1..31 cover each unordered block pair once at
  weight 1; e=0 (symmetric diagonal square) and e=32 (covered from both
  sides) need weight 0.5 — instead of weighting on device, the host
  appends 0.5-pre-scaled copies of those column blocks to the b slab, so
  every device-side reduce has uniform weight 1.

Per row-block: K=4 bf16 matmuls (TensorE) generate fp32 PSUM tiles of M;
abs-row-sums are computed by DVE tensor_reduce(apply_absolute_value) and
ScalarE activation(Abs, accum_out), split across both engines:
  ACT: one [128,2048] job (e1..16), 4-bank PSUM slot
  DVE: two [128,1024] jobs (e17..24, e25..32{scaled}), 2-bank slots
  DVE: per pair of row-blocks one [128,256] job (both e0{scaled})
"""

import numpy as np

B = 8192
P = 128
NCORE = 8
ABLK = 8            # row-blocks per core
BMAIN = 5120        # main slab: 128*(7 + 33)
BCOLS = BMAIN + 2 * ABLK * P   # + e0-scaled and e32-scaled appendices
E0_OFF = BMAIN                 # appendix 1: 0.5*cols[128a .. 128a+128) at E0_OFF+128a
E32_OFF = BMAIN + ABLK * P     # appendix 2: 0.5*cols[128a+4096 ..+128) at E32_OFF+128a
NACC_DVE = 2 * ABLK + ABLK // 2   # 2 jobs per block + 1 per block-pair
NACC_ACT = ABLK

_cache = {}


def _build():
    """Build + compile the Bass module (once per process)."""
    import concourse.bacc as bacc
    import concourse.tile as tile
    import concourse.mybir as mybir

    f32 = mybir.dt.float32
    bf16 = mybir.dt.bfloat16
    nc = bacc.Bacc("TRN2", target_bir_lowering=False, debug=False, num_devices=NCORE)

    a_dram = nc.dram_tensor("a_rows", [4, P * ABLK], bf16, kind="ExternalInput")
    b_dram = nc.dram_tensor("b_cols", [4, BCOLS], bf16, kind="ExternalInput")
    acc_dram = nc.dram_tensor("acc", [P, NACC_DVE + NACC_ACT], f32, kind="ExternalOutput")

    with tile.TileContext(nc) as tc:
        with (
            tc.tile_pool(name="inp", bufs=1) as inp_pool,
            tc.tile_pool(name="accp", bufs=1) as acc_pool,
            tc.tile_pool(name="ps_act", bufs=1, space="PSUM") as ps_act,
            tc.tile_pool(name="ps_dve", bufs=2, space="PSUM") as ps_dve,
        ):
            a_sb = inp_pool.tile([4, P * ABLK], bf16)
            b_sb = inp_pool.tile([4, BCOLS], bf16)
            nc.sync.dma_start(a_sb[:, :], a_dram.ap()[:, :])
            half = BCOLS // 2
            nc.sync.dma_start(b_sb[:, 0:half], b_dram.ap()[:, 0:half])
            nc.sync.dma_start(b_sb[:, half:BCOLS], b_dram.ap()[:, half:BCOLS])

            acc_dve = acc_pool.tile([P, NACC_DVE], f32)
            acc_act = acc_pool.tile([P, NACC_ACT], f32)

            def mm(tile_ap, poff, coff, n):
                nc.tensor.matmul(
                    tile_ap[:, poff:poff + n],
                    lhsT,
                    b_sb[0:4, coff:coff + n],
                    start=True,
                    stop=True,
                )

            dve_col = 0
            for a in range(ABLK):
                base = P * a
                lhsT = a_sb[0:4, base:base + P]

                # --- ACT job: e1..16 -> [128, 2048], 4 matmuls of 512
                t_act = ps_act.tile([P, 2048], f32, tag="act4")
                for g in range(4):
                    mm(t_act, 512 * g, base + 128 + 512 * g, 512)
                nc.scalar.activation(
                    t_act[:, :],
                    t_act[:, :],
                    mybir.ActivationFunctionType.Abs,
                    accum_out=acc_act[:, a:a + 1],
                )

                # --- DVE job 1: e17..24 -> [128, 1024], 2 matmuls of 512
                t_d1 = ps_dve.tile([P, 1024], f32, tag="dve2")
                mm(t_d1, 0, base + 128 * 17, 512)
                mm(t_d1, 512, base + 128 * 21, 512)
                nc.vector.tensor_reduce(
                    acc_dve[:, dve_col:dve_col + 1], t_d1[:, :],
                    axis=mybir.AxisListType.X, op=mybir.AluOpType.add,
                    apply_absolute_value=True,
                )
                dve_col += 1

                # --- DVE job 2: e25..31 + e32(pre-scaled) -> [128, 1024]
                t_d2 = ps_dve.tile([P, 1024], f32, tag="dve2")
                mm(t_d2, 0, base + 128 * 25, 512)
                mm(t_d2, 512, base + 128 * 29, 384)
                mm(t_d2, 896, E32_OFF + base, 128)
                nc.vector.tensor_reduce(
                    acc_dve[:, dve_col:dve_col + 1], t_d2[:, :],
                    axis=mybir.AxisListType.X, op=mybir.AluOpType.add,
                    apply_absolute_value=True,
                )
                dve_col += 1

                # --- per pair: both e0 diag squares (pre-scaled) -> [128, 256]
                if a % 2 == 1:
                    t_d0 = ps_dve.tile([P, 256], f32, tag="dve2")
                    mm(t_d0, 0, E0_OFF + P * (a - 1), 128)
                    mm(t_d0, 128, E0_OFF + P * a, 128)
                    nc.vector.tensor_reduce(
                        acc_dve[:, dve_col:dve_col + 1], t_d0[:, :],
                        axis=mybir.AxisListType.X, op=mybir.AluOpType.add,
                        apply_absolute_value=True,
                    )
                    dve_col += 1

            assert dve_col == NACC_DVE
            nc.sync.dma_start(acc_dram.ap()[:, 0:NACC_DVE], acc_dve[:, :])
            nc.sync.dma_start(acc_dram.ap()[:, NACC_DVE:], acc_act[:, :])

    nc.compile()
    return nc


def _get_nc():
    if "nc" not in _cache:
        _cache["nc"] = _build()
    return _cache["nc"]


def _make_in_maps(p, t, f, u):
    import ml_dtypes

    A = np.ascontiguousarray(
        np.stack([f * u, f, f * p, f * t]).astype(ml_dtypes.bfloat16)
    )
    Bm = np.ascontiguousarray(
        np.stack([f, f * u, -f * t, -f * p]).astype(ml_dtypes.bfloat16)
    )
    Bh = Bm * np.asarray(0.5, dtype=ml_dtypes.bfloat16)  # exact halving

    in_maps = []
    for k in range(NCORE):
        a_rows = np.ascontiguousarray(A[:, 1024 * k:1024 * k + 1024])
        b_cols = np.empty((4, BCOLS), dtype=ml_dtypes.bfloat16)
        cols = (1024 * k + np.arange(BMAIN)) % B
        b_cols[:, 0:BMAIN] = Bm[:, cols]
        e0_cols = (1024 * k + np.arange(ABLK * P)) % B
        b_cols[:, E0_OFF:E0_OFF + ABLK * P] = Bh[:, e0_cols]
        e32_cols = (1024 * k + 4096 + np.arange(ABLK * P)) % B
        b_cols[:, E32_OFF:E32_OFF + ABLK * P] = Bh[:, e32_cols]
        in_maps.append({"a_rows": a_rows, "b_cols": np.ascontiguousarray(b_cols)})
    return in_maps, A, Bm


def kernel(pred, gt, gt_fracTime, gt_ifMOF):
    from concourse import bass_utils

    pred = np.asarray(pred)
    gt = np.asarray(gt)
    ift = int(np.asarray(gt_fracTime))
    imf = int(np.asarray(gt_ifMOF))

    p = pred.astype(np.float32)
    t = gt[:, ift].astype(np.float32)
    f = (gt[:, imf] == 1).astype(np.float32)
    u = (p * t).astype(np.float32)

    in_maps, A, Bm = _make_in_maps(p, t, f, u)
    nc = _get_nc()
    res = bass_utils.run_bass_kernel_spmd(nc, in_maps, core_ids=list(range(NCORE)))

    # T = sum_{i<j} ff |M| (all device accumulator columns are weight 1)
    T = 0.0
    for r in res.results:
        T += r["acc"].astype(np.float64).sum()

    # host closed form in fp64 over the same bf16 values the device used:
    # sum_{i<j} M = (sum_{i,j} M - sum_diag M) / 2
    A64 = A.astype(np.float64)
    B64 = Bm.astype(np.float64)
    S_all = (A64.sum(axis=1) * B64.sum(axis=1)).sum()
    D_diag = (A64 * B64).sum()
    S_half = (S_all - D_diag) / 2.0

    f64 = f.astype(np.float64)
    S_f = f64.sum()
    n_pairs = (S_f * S_f - S_f) / 2.0

    loss = 0.5 * (S_half + T) / 100.0 / n_pairs
    return np.asarray(np.float32(loss))


# revision 7
# speedup vs baseline: 1.0599x; 1.0599x over previous
"""Trainium2 Bass kernel for the pairwise concordance-index loss.

reference:
    loss = sum_{i<j, f_i=f_j=1} relu((p_i-p_j)(t_i-t_j)) / 100 / n_pairs

Math:
  M[i,j] = f_i f_j (p_i-p_j)(t_i-t_j) = A^T B, rank 4:
      A = [f*u, f, f*p, f*t],  B = [f, f*u, -f*t, -f*p],  u = p*t
  (flags fold in because relu(f_i f_j x) = f_i f_j relu(x) for 0/1 flags)
  sum relu(M) = 0.5*(sum M + sum |M|); sum M has an O(B) closed form done
  on the host in fp64; sum |M| is the O(B^2) part done on device.

Device decomposition (8 cores, identical program, data-sharded):
  64 row-blocks of 128 rows; core k owns blocks 8k..8k+7 as two gangs of
  4. Each block processes cyclic column-offsets e=0..32 (cols 128a+128e
  mod 8192): e=1..31 at weight 1; e=0 / e=32 at weight 0.5 via
  0.5-pre-scaled slab appendices (host-side), so all device sums have
  uniform weight.

Device structure per gang (4 row-blocks in lockstep):
  K=4 bf16 matmuls generate M. The 4 blocks' matmuls are packed into
  disjoint 32-row PE groups via tile_position (rows 0/32/64/96) and run
  CONCURRENTLY (~3x PE throughput; K=4 matmuls never warm the HAM clock,
  so concurrency is the only lever). Each "quad" (4 concurrent N<=512
  matmuls) fills the 4 banks of one [128, 4, 512] PSUM tile, which is
  consumed by ONE abs-row-sum job on either the DVE
  (tensor_reduce(apply_absolute_value, axis=XY)) or the ScalarE
  (activation(Abs, accum_out)), alternating to balance both engines.
"""

import numpy as np

B = 8192
P = 128
NCORE = 8
ABLK = 8            # row-blocks per core (2 gangs of 4)
BMAIN = 5120        # main slab: 128*(7 + 33)
BCOLS = BMAIN + 2 * ABLK * P
E0_OFF = BMAIN                 # 0.5*cols[128a ..+128) at E0_OFF+128a
E32_OFF = BMAIN + ABLK * P     # 0.5*cols[128a+4096 ..+128) at E32_OFF+128a

# per gang: Q1..Q7 (N=512 quads), Q8 (N=384 quad), Q9 (e32|e0 two N=128 quads)
NJOBS = 18          # 9 tiles per gang x 2 gangs

_cache = {}


def _build():
    """Build + compile the Bass module (once per process)."""
    import concourse.bacc as bacc
    import concourse.tile as tile
    import concourse.mybir as mybir

    f32 = mybir.dt.float32
    bf16 = mybir.dt.bfloat16
    nc = bacc.Bacc("TRN2", target_bir_lowering=False, debug=False, num_devices=NCORE)

    a_dram = nc.dram_tensor("a_rows", [P, 2 * P], bf16, kind="ExternalInput")
    b_dram = nc.dram_tensor("b_cols", [4, BCOLS], bf16, kind="ExternalInput")
    acc_dram = nc.dram_tensor("acc", [P, NJOBS], f32, kind="ExternalOutput")

    with tile.TileContext(nc) as tc:
        with (
            tc.tile_pool(name="inp", bufs=1) as inp_pool,
            tc.tile_pool(name="accp", bufs=1) as acc_pool,
            tc.tile_pool(name="ps", bufs=2, space="PSUM") as ps,
        ):
            a_sb = inp_pool.tile([P, 2 * P], bf16)
            nc.sync.dma_start(a_sb[:, :], a_dram.ap()[:, :])
            # replicate the 4 B-factor rows into all four 32-row groups
            b_sb = inp_pool.tile([P, BCOLS], bf16)
            for q in range(4):
                nc.sync.dma_start(b_sb[32 * q:32 * q + 4, :], b_dram.ap()[:, :])

            acc_sb = acc_pool.tile([P, NJOBS], f32)

            job = 0
            for g in range(2):          # gangs: row-blocks 4g..4g+3
                # (tile_cols_per_group, [(psum_off, col_off_fn, n), ...])
                def quad(mtile, poff, coff_of_a, n, g=g):
                    """4 concurrent matmuls, one per row-block of the gang."""
                    for q in range(4):
                        coff = coff_of_a(4 * g + q)
                        nc.tensor.matmul(
                            mtile[:, q, poff:poff + n],
                            a_sb[32 * q:32 * q + 4, P * g:P * g + P],
                            b_sb[32 * q:32 * q + 4, coff:coff + n],
                            start=True,
                            stop=True,
                            tile_position=(32 * q, 0),
                        )

                jobs = []
                # Q1..Q7: e = 1..28 in steps of 4 (N=512)
                for s in range(7):
                    jobs.append((512, lambda a, s=s: P * a + 128 * (1 + 4 * s), 512))
                # Q8: e29..31 (N=384)
                jobs.append((512, lambda a: P * a + 128 * 29, 384))
                # Q9: e32 then e0 (both N=128, pre-scaled), share one tile
                jobs.append(None)

                for j, spec in enumerate(jobs):
                    mtile = ps.tile([P, 4, 512], f32, tag="q")
                    if spec is None:
                        quad(mtile, 0, lambda a: E32_OFF + P * a, 128)
                        quad(mtile, 128, lambda a: E0_OFF + P * a, 128)
                        red = mtile[:, :, 0:256]
                    else:
                        _, coff_of_a, n = spec
                        quad(mtile, 0, coff_of_a, n)
                        red = mtile[:, :, 0:n]

                    # alternate engines; Q9 goes to ACT in gang 0, DVE in gang 1
                    if spec is None:
                        use_dve = (g == 1)
                    else:
                        use_dve = (j % 2 == 0)
                    if use_dve:
                        nc.vector.tensor_reduce(
                            acc_sb[:, job:job + 1], red,
                            axis=mybir.AxisListType.XY, op=mybir.AluOpType.add,
                            apply_absolute_value=True,
                        )
                    else:
                        nc.scalar.activation(
                            red, red,
                            mybir.ActivationFunctionType.Abs,
                            accum_out=acc_sb[:, job:job + 1],
                        )
                    job += 1

            assert job == NJOBS
            nc.sync.dma_start(acc_dram.ap()[:, :], acc_sb[:, :])

    nc.compile()
    return nc


def _get_nc():
    if "nc" not in _cache:
        _cache["nc"] = _build()
    return _cache["nc"]


def _make_in_maps(p, t, f, u):
    import ml_dtypes

    A = np.ascontiguousarray(
        np.stack([f * u, f, f * p, f * t]).astype(ml_dtypes.bfloat16)
    )
    Bm = np.ascontiguousarray(
        np.stack([f, f * u, -f * t, -f * p]).astype(ml_dtypes.bfloat16)
    )
    Bh = Bm * np.asarray(0.5, dtype=ml_dtypes.bfloat16)  # exact halving

    in_maps = []
    for k in range(NCORE):
        # a_rows layout: row 32q+r = factor r of row-block 4g+q, cols 128g..+128
        a_rows = np.zeros((P, 2 * P), dtype=ml_dtypes.bfloat16)
        for g in range(2):
            for q in range(4):
                a = 4 * g + q
                rows = slice(1024 * k + P * a, 1024 * k + P * a + P)
                a_rows[32 * q:32 * q + 4, P * g:P * g + P] = A[:, rows]

        b_cols = np.empty((4, BCOLS), dtype=ml_dtypes.bfloat16)
        cols = (1024 * k + np.arange(BMAIN)) % B
        b_cols[:, 0:BMAIN] = Bm[:, cols]
        e0_cols = (1024 * k + np.arange(ABLK * P)) % B
        b_cols[:, E0_OFF:E0_OFF + ABLK * P] = Bh[:, e0_cols]
        e32_cols = (1024 * k + 4096 + np.arange(ABLK * P)) % B
        b_cols[:, E32_OFF:E32_OFF + ABLK * P] = Bh[:, e32_cols]
        in_maps.append(
            {"a_rows": a_rows, "b_cols": np.ascontiguousarray(b_cols)}
        )
    return in_maps, A, Bm


def kernel(pred, gt, gt_fracTime, gt_ifMOF):
    from concourse import bass_utils

    pred = np.asarray(pred)
    gt = np.asarray(gt)
    ift = int(np.asarray(gt_fracTime))
    imf = int(np.asarray(gt_ifMOF))

    p = pred.astype(np.float32)
    t = gt[:, ift].astype(np.float32)
    f = (gt[:, imf] == 1).astype(np.float32)
    u = (p * t).astype(np.float32)

    in_maps, A, Bm = _make_in_maps(p, t, f, u)
    nc = _get_nc()
    res = bass_utils.run_bass_kernel_spmd(nc, in_maps, core_ids=list(range(NCORE)))

    # T = sum_{i<j} ff |M| (all device accumulator columns are weight 1)
    T = 0.0
    for r in res.results:
        T += r["acc"].astype(np.float64).sum()

    # host closed form in fp64 over the same bf16 values the device used:
    # sum_{i<j} M = (sum_{i,j} M - sum_diag M) / 2
    A64 = A.astype(np.float64)
    B64 = Bm.astype(np.float64)
    S_all = (A64.sum(axis=1) * B64.sum(axis=1)).sum()
    D_diag = (A64 * B64).sum()
    S_half = (S_all - D_diag) / 2.0

    f64 = f.astype(np.float64)
    S_f = f64.sum()
    n_pairs = (S_f * S_f - S_f) / 2.0

    loss = 0.5 * (S_half + T) / 100.0 / n_pairs
    return np.asarray(np.float32(loss))
